# revision 1
# baseline (speedup 1.0000x reference)
"""MixIT loss kernel for Trainium2 (raw Bass), 8-way data-parallel over batch.

Math: the loss only depends on the 10x10 Gram matrix of the stacked signals
D = [sources(8); mixtures(2)] over T=32000:
  noise_energy[k,j] = ||x_j - sum_{m in S_kj} s_m||^2
expands into entries of G = D @ D.T.  With a_k = assignment row of mixture 1:
  d1_k = ne1_k + tau*E1 = E1*(1+tau) - 2*a_k.C1 + a_k G a_k
  d0_k = ne0_k + tau*E0 = (E0*(1+tau) - 2*sum(C0) + sum(G)) + 2*a_k.(C0-h) + a_k G a_k
  per_sample = 10/ln(10) * (ln(min_k d1_k*d0_k) - ln(E0*E1))

Dataflow per core (one batch sample per core, host averages 8 scalars):
  1. The host pre-interleaves the sample to R[p, b*100 + s*10 + i] =
     D[s, p*250 + b*10 + i] (pure layout change), so the load DMA moves 128
     fat 10KB partition rows at full HBM bandwidth in 3 overlapping waves
     (small first wave so compute starts early).
  2. The DVE casts each wave f32 -> bf16 (contiguous copy).
  3. 25 bf16 PE matmuls (each 100-column block against itself; every block
     holds all 10 signals interleaved) accumulate a 100x100 f32 PSUM whose
     (i == i') 10x10 sub-grid holds partial Grams; 10 fp32 selector matmuls
     (lhsT = identity[:, i::10]) fold them into G10 -- compute engines
     cannot address partition offset/stride 10, PE contraction can.
     bf16 only rounds the *inputs* (products/accumulation stay f32):
     measured loss error ~3e-5.  The tiny combo stage stays fp32.
  4. Combo: qt = G @ [a | ones] (fp32; the ones column yields h = G.1 for
     free), prod4[8, 508] = [(qt+c1)*a | (qt+c0)*a] via two DVE STTs reading
     qt PSUM directly, two [1,254] matmuls against ones give the noise-energy
     rows; DVE folds scalar terms, takes the min over 254 combos, ACT the log
     (table preloaded early).  Non-critical scalar reductions are scheduled
     after the critical chain so they hide under the PE matmuls.

Raw Bass (not Tile): this toolchain's codegen allows a single sync-wait slot
per instruction, so all cross-engine waits are standalone wait_ge
instructions and each engine runs a hand-scheduled program.  Same-engine RAW
chains also need explicit semaphore handshakes (engines are deep pipelines).
"""

import itertools
from contextlib import ExitStack

import numpy as np

from concourse import bass, mybir
from concourse.bass_utils import run_bass_kernel_spmd

F32 = mybir.dt.float32
BF16 = mybir.dt.bfloat16

B = 8
M = 8  # sources
NMIX = 2
NSIG = M + NMIX  # 10 signals stacked: sources then mixtures
T = 32000
P = 128
NCHUNK = T // P  # 250 elements per partition per signal
LBLK = 10  # i-values per Gram block (10*10 = 100 <= 128 stationary cols)
NBLK = NCHUNK // LBLK  # 25 Gram blocks
BW = NSIG * LBLK  # 100 columns per Gram block
K = 2**M - 2  # 254 assignment combos
TAU = 1e-6
LOG10_SCALE = 10.0 / float(np.log(10.0))

WAVE_EDGES = [0, 5, 15, 25]  # Gram-block ranges per DMA wave
N_WAVES = len(WAVE_EDGES) - 1

CST_COLS = BW + 1 + K + 2  # identity | ones col | a1 | ones col | pad


def _assignment_matrix() -> np.ndarray:
    """[M, K] f32: a1[m, k] = 1 if source m goes to mixture 1 under combo k.

    Same enumeration as the reference's build_A_system (order irrelevant: the
    loss takes a min over k).
    """
    cols = [a for a in itertools.product([0, 1], repeat=M) if 0 < sum(a) < M]
    return np.array(cols, dtype=np.float32).T.copy()


def _const_matrix() -> np.ndarray:
    """[100, 356] f32: [identity(100) | ones10 col | a1 | ones col]."""
    c = np.zeros((BW, CST_COLS), dtype=np.float32)
    c[:BW, :BW] = np.eye(BW, dtype=np.float32)
    c[:M, BW] = 1.0
    c[:M, BW + 1 : BW + 1 + K] = _assignment_matrix()
    c[:M, BW + 1 + K] = 1.0  # ones column -> h; last column stays zero (pad)
    return c


def _interleave(sample: np.ndarray) -> np.ndarray:
    """[NSIG, T] -> [P, NSIG*NCHUNK]: R[p, b*100+s*10+i] = D[s, p*250+b*10+i]."""
    v = sample.reshape(NSIG, P, NBLK, LBLK).transpose(1, 2, 0, 3)
    return np.ascontiguousarray(v).reshape(P, NSIG * NCHUNK)


def _build_kernel() -> bass.Bass:
    nc = bass.Bass(trn_type="TRN2")
    data = nc.declare_dram_parameter("data", [P, NSIG * NCHUNK], F32, isOutput=False)
    cst = nc.declare_dram_parameter("cst", [BW, CST_COLS], F32, isOutput=False)
    out = nc.declare_dram_parameter("loss", [1, 1], F32, isOutput=True)

    with ExitStack() as ctx:
        sb = lambda name, shape, dt=F32: ctx.enter_context(
            nc.sbuf_tensor(name, shape, dt)
        )
        ps = lambda name, shape: ctx.enter_context(nc.psum_tensor(name, shape, F32))

        nat = sb("nat", [P, NSIG * NCHUNK])
        rint = sb("rint", [P, NSIG * NCHUNK], BF16)
        csb = sb("csb", [BW, CST_COLS])
        va1 = sb("va1", [M, K + 2])
        pc = sb("pc", [BW, BW])
        g10 = sb("g10", [NSIG, NSIG])
        rowsb = sb("rowsb", [1, 3 * NSIG])
        sg = sb("sg", [1, 1])
        es2 = sb("es2", [1, 2])  # [E0*(1+tau), E1*(1+tau)]
        s2 = sb("s2", [1, 1])
        s3 = sb("s3", [1, 1])
        e0s = sb("e0s", [1, 1])
        c01m2 = sb("c01m2", [M, 2])  # [-2*C0 | -2*C1]
        c0m2 = sb("c0m2", [M, 1])    # 2*(C0 - h)
        prod = sb("prod", [M, K])
        prod4 = sb("prod4", [M, 2 * K])
        d0c = sb("d0c", [1, K])
        pk = sb("pk", [1, K])
        mn = sb("mn", [1, 1])
        ee = sb("ee", [1, 1])
        lgee = sb("lgee", [1, 1])
        lnee_s = sb("lnee_s", [1, 1])
        d1c = sb("d1c", [1, K])
        lg = sb("lg", [1, 1])
        loss = sb("loss_t", [1, 1])

        gp = ps("gp", [BW, BW])
        g10p = ps("g10p", [NSIG, NSIG])
        rowp = ps("rowp", [1, 3 * NSIG])
        qt = ps("qt", [M, K + 2])  # col K = h = G @ ones, col K+1 pad
        ne1 = ps("ne1", [1, K])
        ne0 = ps("ne0", [1, K])

        dsem_w = [
            ctx.enter_context(nc.semaphore(f"dsem_w{w}")) for w in range(N_WAVES)
        ]
        dsem0 = ctx.enter_context(nc.semaphore("dsem0"))  # cst
        dsem_out = ctx.enter_context(nc.semaphore("dsem_out"))
        pe_sem = ctx.enter_context(nc.semaphore("pe_sem"))
        dve_sem = ctx.enter_context(nc.semaphore("dve_sem"))
        act_sem = ctx.enter_context(nc.semaphore("act_sem"))
        block = ctx.enter_context(nc.Block())

        e0_ap = rowsb[0:1, M : M + 1]                     # G10[8,8]
        e1_ap = rowsb[0:1, NSIG + M + 1 : NSIG + M + 2]   # G10[9,9]
        sumc0_ap = rowsb[0:1, 2 * NSIG + M : 2 * NSIG + M + 1]
        id100 = csb[:, 0:BW]
        ones10 = csb[0:NSIG, BW : BW + 1]
        a1x_ap = csb[0:M, BW + 1 : BW + 1 + K + 2]
        e8col = csb[0:NSIG, M : M + 1]
        e9col = csb[0:NSIG, M + 1 : M + 2]
        h_ap = qt[0:M, K : K + 1]
        e1s_ap = es2[0:1, 1:2]
        s1_ap = es2[0:1, 0:1]

        @block.sync
        def _(sync):
            # data waves first: they are the critical path
            for w in range(N_WAVES):
                c0 = WAVE_EDGES[w] * BW
                c1 = WAVE_EDGES[w + 1] * BW
                sync.dma_start(out=nat[:, c0:c1], in_=data[:, c0:c1]).then_inc(
                    dsem_w[w], 16
                )
            sync.wait_ge(dve_sem, 23)
            sync.dma_start(out=out[:, :], in_=loss[:, :]).then_inc(dsem_out, 16)
            sync.wait_ge(dsem_out, 16)

        @block.vector
        def _(vector):
            # Same-engine RAW needs explicit sem handshakes; waits are only
            # emitted when not already covered by an earlier one.
            for w in range(N_WAVES):
                c0 = WAVE_EDGES[w] * BW
                c1 = WAVE_EDGES[w + 1] * BW
                vector.wait_ge(dsem_w[w], 16)
                vector.tensor_copy(rint[:, c0:c1], nat[:, c0:c1]).then_inc(
                    dve_sem, 1
                )  # 1..3
            vector.wait_ge(dsem0, 16)
            vector.tensor_copy(va1[:, :], a1x_ap).then_inc(dve_sem, 1)        # 4
            vector.wait_ge(pe_sem, NBLK)
            vector.tensor_copy(pc[:, :], gp[:, :]).then_inc(dve_sem, 1)       # 5
            vector.wait_ge(pe_sem, NBLK + LBLK)
            vector.tensor_copy(g10[:, :], g10p[:, :]).then_inc(dve_sem, 1)    # 6
            vector.wait_ge(dve_sem, 6)
            vector.tensor_scalar_mul(
                c01m2[:, :], g10[0:M, M : M + 2], -2.0
            ).then_inc(dve_sem, 1)                                            # 7
            vector.wait_ge(pe_sem, NBLK + LBLK + 1)  # qt (h column)
            vector.wait_ge(dve_sem, 7)
            vector.scalar_tensor_tensor(
                c0m2[:, :], h_ap, -2.0, c01m2[:, 0:1],
                op0=mybir.AluOpType.mult, op1=mybir.AluOpType.subtract,
            ).then_inc(dve_sem, 1)                                            # 8
            vector.tensor_mul(prod[:, :], qt[0:M, 0:K], va1[:, 0:K]).then_inc(
                dve_sem, 1
            )                                                                 # 9
            vector.wait_ge(dve_sem, 9)
            vector.scalar_tensor_tensor(
                prod4[:, K : 2 * K], va1[:, 0:K], c0m2[:, 0:1], prod[:, :],
                op0=mybir.AluOpType.mult, op1=mybir.AluOpType.add,
            ).then_inc(dve_sem, 1)                                            # 10
            vector.scalar_tensor_tensor(
                prod4[:, 0:K], va1[:, 0:K], c01m2[:, 1:2], prod[:, :],
                op0=mybir.AluOpType.mult, op1=mybir.AluOpType.add,
            ).then_inc(dve_sem, 1)                                            # 11
            # ---- non-critical scalar reductions (hide under PE matmuls) ----
            vector.wait_ge(pe_sem, NBLK + LBLK + 4)  # 3 rowp done
            vector.tensor_copy(rowsb[:, :], rowp[:, :]).then_inc(dve_sem, 1)  # 12
            vector.wait_ge(dve_sem, 12)
            vector.reduce_sum(
                sg[:, :],
                rowsb[0:1, 2 * NSIG : 2 * NSIG + M],
                axis=mybir.AxisListType.X,
            ).then_inc(dve_sem, 1)                                            # 13
            vector.tensor_scalar_mul(
                es2[:, :], rowsb[0:1, M : 2 * NSIG : NSIG + 1], 1.0 + TAU
            ).then_inc(dve_sem, 1)                                            # 14
            vector.wait_ge(dve_sem, 14)
            vector.scalar_tensor_tensor(
                s2[:, :], sumc0_ap, -2.0, s1_ap,
                op0=mybir.AluOpType.mult, op1=mybir.AluOpType.add,
            ).then_inc(dve_sem, 1)                                            # 15
            vector.wait_ge(dve_sem, 15)
            vector.tensor_add(e0s[:, :], s2[:, :], sg[:, :]).then_inc(dve_sem, 1)  # 16
            vector.tensor_mul(ee[:, :], e0_ap, e1_ap).then_inc(dve_sem, 1)    # 17
            vector.wait_ge(act_sem, 1)
            vector.tensor_scalar_mul(lnee_s[:, :], lgee[:, :], LOG10_SCALE).then_inc(
                dve_sem, 1
            )                                                                 # 18
            # ---- final combo fold --------------------------------------
            vector.wait_ge(pe_sem, NBLK + LBLK + 5)  # ne0 row
            vector.tensor_scalar_add(
                d0c[:, :], ne0[:, :], e0s[:, :]
            ).then_inc(dve_sem, 1)                                            # 19
            vector.wait_ge(pe_sem, NBLK + LBLK + 6)  # ne1 row
            vector.tensor_scalar_add(
                d1c[:, :], ne1[:, :], e1s_ap
            ).then_inc(dve_sem, 1)                                            # 20
            vector.wait_ge(dve_sem, 20)
            vector.tensor_mul(pk[:, :], d1c[:, :], d0c[:, :]).then_inc(dve_sem, 1)  # 21
            vector.wait_ge(dve_sem, 21)
            vector.tensor_reduce(
                mn[:, :], pk[:, :], axis=mybir.AxisListType.X, op=mybir.AluOpType.min
            ).then_inc(dve_sem, 1)                                            # 22
            vector.wait_ge(act_sem, 2)
            vector.scalar_tensor_tensor(
                loss[:, :], lg[:, :], LOG10_SCALE, lnee_s[:, :],
                op0=mybir.AluOpType.mult, op1=mybir.AluOpType.subtract,
            ).then_inc(dve_sem, 1)                                            # 23

        @block.scalar
        def _(scalar):
            # cst DMA rides the ACT HW-DGE ring so the data waves own the SP
            # ring; the dummy activation preloads the Ln table (~1.3us) off
            # the critical path; ln(ee) is computed early, ln(mn) at the end.
            scalar.dma_start(out=csb[:, :], in_=cst[:, :]).then_inc(dsem0, 16)
            scalar.wait_ge(dsem0, 16)
            scalar.activation(s3[:, :], csb[0:1, 0:1], mybir.ActivationFunctionType.Ln)
            scalar.wait_ge(dve_sem, 17)
            scalar.activation(
                lgee[:, :], ee[:, :], mybir.ActivationFunctionType.Ln
            ).then_inc(act_sem, 1)
            scalar.wait_ge(dve_sem, 22)
            scalar.activation(
                lg[:, :], mn[:, :], mybir.ActivationFunctionType.Ln
            ).then_inc(act_sem, 1)

        @block.tensor
        def _(tensor):
            for w in range(N_WAVES):
                b0, b1 = WAVE_EDGES[w], WAVE_EDGES[w + 1]
                tensor.wait_ge(dve_sem, w + 1)
                for blk in range(b0, b1):
                    cols = rint[:, blk * BW : (blk + 1) * BW]
                    tensor.matmul(
                        gp[:, :],
                        cols,
                        cols,
                        start=(blk == 0),
                        stop=(blk == NBLK - 1),
                    ).then_inc(pe_sem, 1)
            tensor.wait_ge(dsem0, 16)  # consts (identity, ones, a1) landed
            tensor.wait_ge(dve_sem, 5)  # pc copied
            for i in range(LBLK):
                tensor.matmul(
                    g10p[:, :],
                    id100[:, i :: LBLK],
                    pc[:, i :: LBLK],
                    start=(i == 0),
                    stop=(i == LBLK - 1),
                ).then_inc(pe_sem, 1)
            tensor.wait_ge(dve_sem, 6)  # g10 copied (va1 at 4 also covered)
            tensor.matmul(qt[:, :], g10[0:M, 0:M], va1[:, :]).then_inc(pe_sem, 1)
            tensor.matmul(rowp[0:1, 0:NSIG], e8col, g10[:, :]).then_inc(pe_sem, 1)
            tensor.matmul(
                rowp[0:1, NSIG : 2 * NSIG], e9col, g10[:, :]
            ).then_inc(pe_sem, 1)
            tensor.matmul(
                rowp[0:1, 2 * NSIG : 3 * NSIG], ones10[:, :], g10[:, :]
            ).then_inc(pe_sem, 1)
            tensor.wait_ge(dve_sem, 10)  # prod4 ne0-half ready
            tensor.matmul(
                ne0[:, :], ones10[0:M, :], prod4[:, K : 2 * K]
            ).then_inc(pe_sem, 1)
            tensor.wait_ge(dve_sem, 11)  # prod4 ne1-half ready
            tensor.matmul(
                ne1[:, :], ones10[0:M, :], prod4[:, 0:K]
            ).then_inc(pe_sem, 1)

    return nc


_NC_CACHE: bass.Bass | None = None


def kernel(estimated_sources: np.ndarray, input_mixtures: np.ndarray) -> np.ndarray:
    global _NC_CACHE
    assert estimated_sources.shape == (B, M, T)
    assert input_mixtures.shape == (B, NMIX, T)
    if _NC_CACHE is None:
        _NC_CACHE = _build_kernel()
    nc = _NC_CACHE

    cst = _const_matrix()
    est = np.asarray(estimated_sources, dtype=np.float32)
    mx = np.asarray(input_mixtures, dtype=np.float32)
    in_maps = [
        {
            "data": _interleave(np.concatenate([est[b], mx[b]], axis=0)),
            "cst": cst,
        }
        for b in range(B)
    ]
    res = run_bass_kernel_spmd(nc, in_maps, core_ids=list(range(B)))
    vals = np.array([res.results[b]["loss"][0, 0] for b in range(B)], dtype=np.float32)
    return np.asarray(vals.mean(), dtype=np.float32)



# revision 10
# speedup vs baseline: 1.1470x; 1.1470x over previous
"""MixIT loss kernel for Trainium2 (raw Bass), 8-way data-parallel over batch.

Math: the loss only depends on the 10x10 Gram matrix of the stacked signals
D = [sources(8); mixtures(2)] over T=32000:
  d1_k = ne1_k + tau*E1 = S1 - 2*(C1.a_k) + a_k G a_k,    S1 = E1*(1+tau)
  d0_k = ne0_k + tau*E0 = S0 + a_k.(2C0-2h) + a_k G a_k,  S0 = E0*(1+tau) - 2*sumC0 + sumG
  per_sample = 10/ln(10) * (ln(min_k d1_k*d0_k) - ln(E0*E1))

Dataflow per core (one batch sample per core, host averages 8 scalars):
  1. The host interleaves the sample to R[p, b*100 + i*10 + s] =
     D[s, p*250 + b*10 + i] and casts to bf16 (halves HBM bytes; no on-device
     cast stage).  Three DMA waves (12/12/1 Gram blocks) on the SP HW-DGE
     ring; the tiny last wave means only ~100ns of matmul remains after the
     last wave's ~900ns DMA-semaphore latency.  The constant matrix rides the
     same ring after the data so its descriptors overlap the data transfer.
  2. 25 bf16 PE matmuls (each 100-column block against itself) accumulate a
     100x100 f32 PSUM Gram; 10 selector matmuls (contiguous identity slices,
     s-fastest interleave) fold the block-diagonal into G10.
  3. Combo stage, minimal serial chain (all APs partition-0-based; compute
     engines cannot address partition offsets other than 0/32/64/96):
       qte[10, K+2] = G10^T @ [a1(8 rows, zero-padded to 10) | 2*ones]
     one matmul yields qt = G8 a1 (rows 0-7) and 2h in the ones column.
     The linear terms distribute into the per-source products:
       d0 - S0 = sum_s a1_sk (qt_sk - (2h_s - 2C0_s))
       d1 - S1 = sum_s a1_sk (qt_sk - 2C1_s)
     so two [8,1] DVE ops build v0 = 2h-2C0, v1 = 2C1 (from the qte ones
     column and G10 columns 8/9), two STT ops fill buf8[8, 2K] =
     [(qt-v0) o a1 | (qt-v1) o a1], and ones8^T @ buf8 -> ne2[1, 2K] =
     [d0-S0 | d1-S1] in one matmul; then DVE: +S1, (.+S0)*., min, ACT Ln,
     scale-subtract.  The scalar terms S0/S1/ln(E0E1) come from 3 tiny row
     matmuls + DVE reductions scheduled to hide under the combo matmuls; the
     ACT Ln table is preloaded at ~6.5us via a dummy activation on a memset
     scratch.

Raw Bass: single sync-wait slot per instruction, so cross-engine waits are
standalone wait_ge and each engine runs a hand-scheduled in-order program.
"""

import itertools
from contextlib import ExitStack

import ml_dtypes
import numpy as np

from concourse import bass, mybir
from concourse.bass_utils import run_bass_kernel_spmd

F32 = mybir.dt.float32
BF16 = mybir.dt.bfloat16

B = 8
M = 8  # sources
NMIX = 2
NSIG = M + NMIX  # 10 signals stacked: sources then mixtures
T = 32000
P = 128
NCHUNK = T // P  # 250 elements per partition per signal
LBLK = 10  # i-values per Gram block (10*10 = 100 <= 128 stationary cols)
NBLK = NCHUNK // LBLK  # 25 Gram blocks
BW = NSIG * LBLK  # 100 columns per Gram block
K = 2**M - 2  # 254 assignment combos
TAU = 1e-6
LOG10_SCALE = 10.0 / float(np.log(10.0))

WAVE_EDGES = [0, 12, 24, 25]  # Gram-block ranges per DMA wave
N_WAVES = len(WAVE_EDGES) - 1

# cst columns: identity(100) | va1e (K+2) | ones10 | e8 | e9 | ones8
A1OFF = BW
ONES10C = BW + K + 2
E8C = ONES10C + 1
E9C = ONES10C + 2
ONES8C = ONES10C + 3
CST_COLS = ONES10C + 4


def _assignment_matrix() -> np.ndarray:
    """[M, K] f32: a1[m, k] = 1 if source m goes to mixture 1 under combo k."""
    cols = [a for a in itertools.product([0, 1], repeat=M) if 0 < sum(a) < M]
    return np.array(cols, dtype=np.float32).T.copy()


def _const_matrix() -> np.ndarray:
    c = np.zeros((BW, CST_COLS), dtype=np.float32)
    c[:BW, :BW] = np.eye(BW, dtype=np.float32)
    c[:M, A1OFF : A1OFF + K] = _assignment_matrix()
    c[:M, A1OFF + K] = 2.0  # doubled-ones column -> qte[:, K] = 2h
    c[:NSIG, ONES10C] = 1.0
    c[M, E8C] = 1.0
    c[M + 1, E9C] = 1.0
    c[:M, ONES8C] = 1.0
    return c


def _interleave(sample: np.ndarray) -> np.ndarray:
    """[NSIG, T] f32 -> [P, NSIG*NCHUNK] bf16, R[p, b*100+i*10+s] = D[s, p*250+b*10+i]."""
    v = sample.reshape(NSIG, P, NBLK, LBLK).transpose(1, 2, 3, 0)
    return np.ascontiguousarray(v).reshape(P, NSIG * NCHUNK).astype(ml_dtypes.bfloat16)


def _build_kernel() -> bass.Bass:
    nc = bass.Bass(trn_type="TRN2")
    data = nc.declare_dram_parameter("data", [P, NSIG * NCHUNK], BF16, isOutput=False)
    cst = nc.declare_dram_parameter("cst", [BW, CST_COLS], F32, isOutput=False)
    out = nc.declare_dram_parameter("loss", [1, 1], F32, isOutput=True)

    with ExitStack() as ctx:
        sb = lambda name, shape, dt=F32: ctx.enter_context(
            nc.sbuf_tensor(name, shape, dt)
        )
        ps = lambda name, shape: ctx.enter_context(nc.psum_tensor(name, shape, F32))

        rint = sb("rint", [P, NSIG * NCHUNK], BF16)
        csb = sb("csb", [BW, CST_COLS])
        pc = sb("pc", [BW, BW])
        g10 = sb("g10", [NSIG, NSIG])
        buf8 = sb("buf8", [M, 2 * K])
        v0 = sb("v0", [M, 1])
        v1 = sb("v1", [M, 1])
        rowsb = sb("rowsb", [1, 3 * NSIG])
        dum0 = sb("dum0", [1, 1])
        dum1 = sb("dum1", [1, 1])
        es2 = sb("es2", [1, 2])  # [E0*(1+tau), E1*(1+tau)]
        sg = sb("sg", [1, 1])
        s2 = sb("s2", [1, 1])
        e0s = sb("e0s", [1, 1])
        ee = sb("ee", [1, 1])
        lgee = sb("lgee", [1, 1])
        lnee_s = sb("lnee_s", [1, 1])
        t1 = sb("t1", [1, K])
        pk = sb("pk", [1, K])
        mn = sb("mn", [1, 1])
        lg = sb("lg", [1, 1])
        loss = sb("loss_t", [1, 1])

        gp = ps("gp", [BW, BW])
        g10p = ps("g10p", [NSIG, NSIG])
        qte = ps("qte", [NSIG, K + 2])
        rowp = ps("rowp", [1, 3 * NSIG])
        ne2 = ps("ne2", [1, 2 * K])

        dsem_w = [
            ctx.enter_context(nc.semaphore(f"dsem_w{w}")) for w in range(N_WAVES)
        ]
        dsem_c = ctx.enter_context(nc.semaphore("dsem_c"))
        dsem_out = ctx.enter_context(nc.semaphore("dsem_out"))
        pe_sem = ctx.enter_context(nc.semaphore("pe_sem"))
        dve_sem = ctx.enter_context(nc.semaphore("dve_sem"))
        act_sem = ctx.enter_context(nc.semaphore("act_sem"))
        block = ctx.enter_context(nc.Block())

        id100 = csb[:, 0:BW]
        a1sb = csb[0:M, A1OFF : A1OFF + K]
        va1e = csb[0:NSIG, A1OFF : A1OFF + K + 2]
        ones10c = csb[0:NSIG, ONES10C : ONES10C + 1]
        e8col = csb[0:NSIG, E8C : E8C + 1]
        e9col = csb[0:NSIG, E9C : E9C + 1]
        ones8c = csb[0:NSIG, ONES8C : ONES8C + 1]

        @block.sync
        def _(sync):
            for w in range(N_WAVES):
                c0 = WAVE_EDGES[w] * BW
                c1 = WAVE_EDGES[w + 1] * BW
                sync.dma_start(out=rint[:, c0:c1], in_=data[:, c0:c1]).then_inc(
                    dsem_w[w], 16
                )
            sync.dma_start(out=csb[:, :], in_=cst[:, :]).then_inc(dsem_c, 16)
            sync.wait_ge(dve_sem, 18)
            sync.dma_start(out=out[:, :], in_=loss[:, :]).then_inc(dsem_out, 16)
            sync.wait_ge(dsem_out, 16)

        @block.vector
        def _(vector):
            vector.memset(dum0[:, :], 1.0).then_inc(dve_sem, 1)                # 1
            vector.wait_ge(pe_sem, NBLK)
            vector.tensor_copy(pc[:, :], gp[:, :]).then_inc(dve_sem, 1)        # 2
            vector.wait_ge(pe_sem, NBLK + LBLK)
            vector.tensor_copy(g10[:, :], g10p[:, :]).then_inc(dve_sem, 1)     # 3
            vector.wait_ge(pe_sem, NBLK + LBLK + 1)  # qte
            vector.wait_ge(dve_sem, 3)
            vector.scalar_tensor_tensor(
                v0[:, :], g10[0:M, M : M + 1], -2.0, qte[0:M, K : K + 1],
                op0=mybir.AluOpType.mult, op1=mybir.AluOpType.add,
            ).then_inc(dve_sem, 1)                                             # 4
            vector.tensor_scalar_mul(
                v1[:, :], g10[0:M, M + 1 : M + 2], 2.0
            ).then_inc(dve_sem, 1)                                             # 5
            vector.wait_ge(dve_sem, 5)
            vector.scalar_tensor_tensor(
                buf8[:, 0:K], qte[0:M, 0:K], v0[:, :], a1sb,
                op0=mybir.AluOpType.subtract, op1=mybir.AluOpType.mult,
            ).then_inc(dve_sem, 1)                                             # 6
            vector.scalar_tensor_tensor(
                buf8[:, K : 2 * K], qte[0:M, 0:K], v1[:, :], a1sb,
                op0=mybir.AluOpType.subtract, op1=mybir.AluOpType.mult,
            ).then_inc(dve_sem, 1)                                             # 7
            # ---- scalar terms (hide under the combo matmuls) ------------
            vector.wait_ge(pe_sem, NBLK + LBLK + 4)  # rowp x3 done
            vector.tensor_copy(rowsb[:, :], rowp[:, :]).then_inc(dve_sem, 1)   # 8
            vector.wait_ge(dve_sem, 8)
            vector.tensor_scalar_mul(
                es2[:, :], rowsb[0:1, M : 2 * NSIG : NSIG + 1], 1.0 + TAU
            ).then_inc(dve_sem, 1)                                             # 9
            vector.reduce_sum(
                sg[:, :], rowsb[0:1, 2 * NSIG : 2 * NSIG + M],
                axis=mybir.AxisListType.X,
            ).then_inc(dve_sem, 1)                                             # 10
            vector.tensor_mul(
                ee[:, :], rowsb[0:1, M : M + 1], rowsb[0:1, 2 * NSIG - 1 : 2 * NSIG]
            ).then_inc(dve_sem, 1)                                             # 11
            vector.wait_ge(dve_sem, 11)
            vector.scalar_tensor_tensor(
                s2[:, :], rowsb[0:1, 3 * NSIG - 2 : 3 * NSIG - 1], -2.0,
                es2[0:1, 0:1],
                op0=mybir.AluOpType.mult, op1=mybir.AluOpType.add,
            ).then_inc(dve_sem, 1)                                             # 12
            vector.wait_ge(dve_sem, 12)
            vector.tensor_add(e0s[:, :], s2[:, :], sg[:, :]).then_inc(dve_sem, 1)  # 13
            # ---- final combo fold ---------------------------------------
            vector.wait_ge(pe_sem, NBLK + LBLK + 5)  # ne2
            vector.tensor_scalar_add(
                t1[:, :], ne2[0:1, K : 2 * K], es2[0:1, 1:2]
            ).then_inc(dve_sem, 1)                                             # 14
            vector.wait_ge(dve_sem, 14)
            vector.scalar_tensor_tensor(
                pk[:, :], ne2[0:1, 0:K], e0s[0:1, 0:1], t1[:, :],
                op0=mybir.AluOpType.add, op1=mybir.AluOpType.mult,
            ).then_inc(dve_sem, 1)                                             # 15
            vector.wait_ge(dve_sem, 15)
            vector.tensor_reduce(
                mn[:, :], pk[:, :], axis=mybir.AxisListType.X,
                op=mybir.AluOpType.min,
            ).then_inc(dve_sem, 1)                                             # 16
            vector.wait_ge(act_sem, 1)
            vector.tensor_scalar_mul(
                lnee_s[:, :], lgee[:, :], LOG10_SCALE
            ).then_inc(dve_sem, 1)                                             # 17
            vector.wait_ge(dve_sem, 17)
            vector.wait_ge(act_sem, 2)
            vector.scalar_tensor_tensor(
                loss[:, :], lg[:, :], LOG10_SCALE, lnee_s[:, :],
                op0=mybir.AluOpType.mult, op1=mybir.AluOpType.subtract,
            ).then_inc(dve_sem, 1)                                             # 18

        @block.scalar
        def _(scalar):
            # dummy activation on memset scratch preloads the Ln table early
            scalar.wait_ge(dve_sem, 1)
            scalar.activation(dum1[:, :], dum0[:, :], mybir.ActivationFunctionType.Ln)
            scalar.wait_ge(dve_sem, 11)
            scalar.activation(
                lgee[:, :], ee[:, :], mybir.ActivationFunctionType.Ln
            ).then_inc(act_sem, 1)
            scalar.wait_ge(dve_sem, 16)
            scalar.activation(
                lg[:, :], mn[:, :], mybir.ActivationFunctionType.Ln
            ).then_inc(act_sem, 1)

        @block.tensor
        def _(tensor):
            for w in range(N_WAVES):
                b0, b1 = WAVE_EDGES[w], WAVE_EDGES[w + 1]
                tensor.wait_ge(dsem_w[w], 16)
                for blk in range(b0, b1):
                    cols = rint[:, blk * BW : (blk + 1) * BW]
                    tensor.matmul(
                        gp[:, :],
                        cols,
                        cols,
                        start=(blk == 0),
                        stop=(blk == NBLK - 1),
                    ).then_inc(pe_sem, 1)
            tensor.wait_ge(dsem_c, 16)
            tensor.wait_ge(dve_sem, 2)  # pc copied
            for i in range(LBLK):
                tensor.matmul(
                    g10p[:, :],
                    id100[:, i * LBLK : (i + 1) * LBLK],
                    pc[:, i * LBLK : (i + 1) * LBLK],
                    start=(i == 0),
                    stop=(i == LBLK - 1),
                ).then_inc(pe_sem, 1)
            tensor.wait_ge(dve_sem, 3)  # g10 copied
            tensor.matmul(qte[:, :], g10[:, :], va1e).then_inc(pe_sem, 1)
            tensor.matmul(rowp[0:1, 0:NSIG], e8col, g10[:, :]).then_inc(pe_sem, 1)
            tensor.matmul(
                rowp[0:1, NSIG : 2 * NSIG], e9col, g10[:, :]
            ).then_inc(pe_sem, 1)
            tensor.matmul(
                rowp[0:1, 2 * NSIG : 3 * NSIG], ones8c, g10[:, :]
            ).then_inc(pe_sem, 1)
            tensor.wait_ge(dve_sem, 7)  # buf8 ready
            tensor.matmul(
                ne2[:, :], csb[0:M, ONES8C : ONES8C + 1], buf8[:, :]
            ).then_inc(pe_sem, 1)

    return nc


_NC_CACHE: bass.Bass | None = None


def _in_maps(est: np.ndarray, mx: np.ndarray) -> list[dict]:
    cst = _const_matrix()
    return [
        {
            "data": _interleave(np.concatenate([est[b], mx[b]], axis=0)),
            "cst": cst,
        }
        for b in range(B)
    ]


def kernel(estimated_sources: np.ndarray, input_mixtures: np.ndarray) -> np.ndarray:
    global _NC_CACHE
    assert estimated_sources.shape == (B, M, T)
    assert input_mixtures.shape == (B, NMIX, T)
    if _NC_CACHE is None:
        _NC_CACHE = _build_kernel()
    nc = _NC_CACHE

    est = np.asarray(estimated_sources, dtype=np.float32)
    mx = np.asarray(input_mixtures, dtype=np.float32)
    res = run_bass_kernel_spmd(nc, _in_maps(est, mx), core_ids=list(range(B)))
    vals = np.array([res.results[b]["loss"][0, 0] for b in range(B)], dtype=np.float32)
    return np.asarray(vals.mean(), dtype=np.float32)


# revision 18
# speedup vs baseline: 1.2555x; 1.0946x over previous
"""MixIT loss kernel for Trainium2 (raw Bass), 8-way data-parallel over batch.

Math: the loss only depends on the 10x10 Gram matrix of the stacked signals
D = [sources(8); mixtures(2)] over T=32000:
  d1_k = ne1_k + tau*E1 = S1 - 2*(C1.a_k) + a_k G a_k,    S1 = E1*(1+tau)
  d0_k = ne0_k + tau*E0 = S0 + a_k.(2C0-2h) + a_k G a_k,  S0 = E0*(1+tau) - 2*sumC0 + sumG
  per_sample = 10/ln(10) * (ln(min_k d1_k*d0_k) - ln(E0*E1))

Dataflow per core (one batch sample per core, host averages 8 scalars):
  1. The host interleaves the sample to R[p, b*100 + i*10 + s] =
     D[s, p*250 + b*10 + i] and casts to bf16 (halves HBM bytes; no on-device
     cast stage).  Three DMA waves (12/12/1 Gram blocks) on the SP HW-DGE
     ring; the tiny last wave means only ~100ns of matmul remains after the
     last wave's ~900ns DMA-semaphore latency.  The constant matrix rides the
     same ring after the data so its descriptors overlap the data transfer.
  2. 25 bf16 PE matmuls (each 100-column block against itself) accumulate a
     100x100 f32 PSUM Gram; 10 selector matmuls (contiguous identity slices,
     s-fastest interleave) fold the block-diagonal into G10.
  3. Combo stage, minimal serial chain (all APs partition-0-based; compute
     engines cannot address partition offsets other than 0/32/64/96):
       qte[10, K+2] = G10^T @ [a1(8 rows, zero-padded to 10) | 2*ones]
     one matmul yields qt = G8 a1 (rows 0-7) and 2h in the ones column.
     The linear terms distribute into the per-source products:
       d0 - S0 = sum_s a1_sk (qt_sk - (2h_s - 2C0_s))
       d1 - S1 = sum_s a1_sk (qt_sk - 2C1_s)
     so two [8,1] DVE ops build v0 = 2h-2C0, v1 = 2C1 (from the qte ones
     column and G10 columns 8/9), two STT ops fill buf8[8, 2K] =
     [(qt-v0) o a1 | (qt-v1) o a1], and ones8^T @ buf8 -> ne2[1, 2K] =
     [d0-S0 | d1-S1] in one matmul; then DVE: +S1, (.+S0)*., min, ACT Ln,
     scale-subtract.  The scalar terms S0/S1/ln(E0E1) come from 3 tiny row
     matmuls + DVE reductions scheduled to hide under the combo matmuls; the
     ACT Ln table is preloaded at ~6.5us via a dummy activation on a memset
     scratch.

Raw Bass: single sync-wait slot per instruction, so cross-engine waits are
standalone wait_ge and each engine runs a hand-scheduled in-order program.
"""

import itertools
from contextlib import ExitStack

import ml_dtypes
import numpy as np

from concourse import bass, mybir
from concourse.bass_utils import run_bass_kernel_spmd

F32 = mybir.dt.float32
BF16 = mybir.dt.bfloat16

B = 8
M = 8  # sources
NMIX = 2
NSIG = M + NMIX  # 10 signals stacked: sources then mixtures
T = 32000
P = 128
NCHUNK = T // P  # 250 elements per partition per signal
LBLK = 10  # i-values per Gram block (10*10 = 100 <= 128 stationary cols)
NBLK = NCHUNK // LBLK  # 25 Gram blocks
BW = NSIG * LBLK  # 100 columns per Gram block
K = 2**M - 2  # 254 assignment combos
TAU = 1e-6
LOG10_SCALE = 10.0 / float(np.log(10.0))

WAVE_EDGES = [0, 4, 14, 24, 25]  # Gram-block ranges per DMA wave
N_WAVES = len(WAVE_EDGES) - 1

# cst columns: identity(100) | va1e (K+2) | ones10 | e8 | e9 | ones8
A1OFF = BW
ONES10C = BW + K + 2
E8C = ONES10C + 1
E9C = ONES10C + 2
ONES8C = ONES10C + 3
CST_COLS = ONES10C + 4


def _assignment_matrix() -> np.ndarray:
    """[M, K] f32: a1[m, k] = 1 if source m goes to mixture 1 under combo k."""
    cols = [a for a in itertools.product([0, 1], repeat=M) if 0 < sum(a) < M]
    return np.array(cols, dtype=np.float32).T.copy()


def _const_matrix() -> np.ndarray:
    c = np.zeros((BW, CST_COLS), dtype=np.float32)
    c[:BW, :BW] = np.eye(BW, dtype=np.float32)
    c[:M, A1OFF : A1OFF + K] = _assignment_matrix()
    c[:M, A1OFF + K] = 2.0  # doubled-ones column -> qte[:, K] = 2h
    c[:NSIG, ONES10C] = 1.0
    c[M, E8C] = 1.0
    c[M + 1, E9C] = 1.0
    c[:M, ONES8C] = 1.0
    return c


def _interleave(sample: np.ndarray) -> np.ndarray:
    """[NSIG, T] f32 -> [P, NSIG*NCHUNK] bf16, R[p, b*100+i*10+s] = D[s, p*250+b*10+i]."""
    v = sample.reshape(NSIG, P, NBLK, LBLK).transpose(1, 2, 3, 0)
    return np.ascontiguousarray(v).reshape(P, NSIG * NCHUNK).astype(ml_dtypes.bfloat16)


def _build_kernel() -> bass.Bass:
    nc = bass.Bass(trn_type="TRN2")
    data = nc.declare_dram_parameter("data", [P, NSIG * NCHUNK], BF16, isOutput=False)
    cst = nc.declare_dram_parameter("cst", [BW, CST_COLS], F32, isOutput=False)
    out = nc.declare_dram_parameter("loss", [1, 1], F32, isOutput=True)

    with ExitStack() as ctx:
        sb = lambda name, shape, dt=F32: ctx.enter_context(
            nc.sbuf_tensor(name, shape, dt)
        )
        ps = lambda name, shape: ctx.enter_context(nc.psum_tensor(name, shape, F32))

        rint = sb("rint", [P, NSIG * NCHUNK], BF16)
        csb = sb("csb", [BW, CST_COLS])
        csbb = sb("csbb", [NSIG, K + 3], BF16)  # bf16 [va1e | ones8 col]
        pc = sb("pc", [BW, BW])
        g10b = sb("g10b", [NSIG, NSIG], BF16)
        g10 = sb("g10", [NSIG, NSIG])
        buf8 = sb("buf8", [M, 2 * K], BF16)
        v01 = sb("v01", [M, 2])
        rowsb = sb("rowsb", [1, 3 * NSIG])
        dum0 = sb("dum0", [1, 1])
        dum1 = sb("dum1", [1, 1])
        es2 = sb("es2", [1, 2])  # [E0*(1+tau), E1*(1+tau)]
        sg = sb("sg", [1, 1])
        s2 = sb("s2", [1, 1])
        e0s = sb("e0s", [1, 1])
        ee = sb("ee", [1, 1])
        lgee = sb("lgee", [1, 1])
        lnee_s = sb("lnee_s", [1, 1])
        d0c = sb("d0c", [1, K])
        t1 = sb("t1", [1, K])
        pk = sb("pk", [1, K])
        mn = sb("mn", [1, 1])
        lg = sb("lg", [1, 1])
        loss = sb("loss_t", [1, 1])

        gp = ps("gp", [BW, BW])
        g10p = ps("g10p", [NSIG, NSIG])
        qte = ps("qte", [NSIG, K + 2])
        rowp = ps("rowp", [1, 3 * NSIG])
        ne2 = ps("ne2", [1, 2 * K])

        dsem_w = [
            ctx.enter_context(nc.semaphore(f"dsem_w{w}")) for w in range(N_WAVES)
        ]
        dsem_c = ctx.enter_context(nc.semaphore("dsem_c"))
        dsem_out = ctx.enter_context(nc.semaphore("dsem_out"))
        pe_sem = ctx.enter_context(nc.semaphore("pe_sem"))
        dve_sem = ctx.enter_context(nc.semaphore("dve_sem"))
        act_sem = ctx.enter_context(nc.semaphore("act_sem"))
        block = ctx.enter_context(nc.Block())

        id100 = csb[:, 0:BW]
        a1sb = csb[0:M, A1OFF : A1OFF + K]
        va1e = csb[0:NSIG, A1OFF : A1OFF + K + 2]
        ones10c = csb[0:NSIG, ONES10C : ONES10C + 1]
        e8col = csb[0:NSIG, E8C : E8C + 1]
        e9col = csb[0:NSIG, E9C : E9C + 1]
        ones8c = csb[0:NSIG, ONES8C : ONES8C + 1]

        @block.sync
        def _(sync):
            for w in range(N_WAVES):
                c0 = WAVE_EDGES[w] * BW
                c1 = WAVE_EDGES[w + 1] * BW
                sync.dma_start(out=rint[:, c0:c1], in_=data[:, c0:c1]).then_inc(
                    dsem_w[w], 16
                )
            sync.dma_start(out=csb[:, :], in_=cst[:, :]).then_inc(dsem_c, 16)
            sync.wait_ge(dve_sem, 20)
            # No wait on dsem_out: the DMA lands ~7ns after issue while the
            # block-exit barrier + engine drains take ~1.4us after this point,
            # so the store is long complete before the NEFF retires.
            sync.dma_start(out=out[:, :], in_=loss[:, :]).then_inc(dsem_out, 16)

        @block.vector
        def _(vector):
            vector.memset(dum0[:, :], 1.0).then_inc(dve_sem, 1)                # 1
            vector.memset(csbb[0:M, K + 2 : K + 3], 1.0).then_inc(dve_sem, 1)  # 2
            vector.wait_ge(dsem_c, 16)
            vector.tensor_copy(csbb[:, 0 : K + 2], va1e).then_inc(dve_sem, 1)  # 3
            vector.wait_ge(pe_sem, NBLK)
            vector.tensor_copy(pc[:, :], gp[:, :]).then_inc(dve_sem, 1)        # 4
            vector.wait_ge(pe_sem, NBLK + LBLK)
            vector.tensor_copy(g10b[:, :], g10p[:, :]).then_inc(dve_sem, 1)    # 5
            vector.tensor_copy(g10[:, :], g10p[:, :]).then_inc(dve_sem, 1)     # 6
            vector.wait_ge(pe_sem, NBLK + LBLK + 1)  # qte
            vector.wait_ge(dve_sem, 6)
            vector.scalar_tensor_tensor(
                v01[:, :], g10[0:M, M : M + 2], -2.0, qte[0:M, K : K + 2],
                op0=mybir.AluOpType.mult, op1=mybir.AluOpType.add,
            ).then_inc(dve_sem, 1)                                             # 7
            vector.wait_ge(dve_sem, 7)
            vector.scalar_tensor_tensor(
                buf8[:, 0:K], qte[0:M, 0:K], v01[:, 0:1], a1sb,
                op0=mybir.AluOpType.subtract, op1=mybir.AluOpType.mult,
            ).then_inc(dve_sem, 1)                                             # 8
            vector.scalar_tensor_tensor(
                buf8[:, K : 2 * K], qte[0:M, 0:K], v01[:, 1:2], a1sb,
                op0=mybir.AluOpType.add, op1=mybir.AluOpType.mult,
            ).then_inc(dve_sem, 1)                                             # 9
            # ---- scalar terms (hide under the combo matmuls) ------------
            vector.wait_ge(pe_sem, NBLK + LBLK + 4)  # rowp x3 done
            vector.tensor_copy(rowsb[:, :], rowp[:, :]).then_inc(dve_sem, 1)   # 10
            vector.wait_ge(dve_sem, 10)
            vector.tensor_scalar_mul(
                es2[:, :], rowsb[0:1, M : 2 * NSIG : NSIG + 1], 1.0 + TAU
            ).then_inc(dve_sem, 1)                                             # 11
            vector.reduce_sum(
                sg[:, :], rowsb[0:1, 2 * NSIG : 2 * NSIG + M],
                axis=mybir.AxisListType.X,
            ).then_inc(dve_sem, 1)                                             # 12
            vector.tensor_mul(
                ee[:, :], rowsb[0:1, M : M + 1], rowsb[0:1, 2 * NSIG - 1 : 2 * NSIG]
            ).then_inc(dve_sem, 1)                                             # 13
            vector.wait_ge(dve_sem, 13)
            vector.scalar_tensor_tensor(
                s2[:, :], rowsb[0:1, 3 * NSIG - 2 : 3 * NSIG - 1], -2.0,
                es2[0:1, 0:1],
                op0=mybir.AluOpType.mult, op1=mybir.AluOpType.add,
            ).then_inc(dve_sem, 1)                                             # 14
            vector.wait_ge(dve_sem, 14)
            vector.tensor_add(e0s[:, :], s2[:, :], sg[:, :]).then_inc(dve_sem, 1)  # 15
            # ---- final combo fold ---------------------------------------
            vector.wait_ge(pe_sem, NBLK + LBLK + 5)  # ne2
            vector.wait_ge(dve_sem, 15)
            vector.tensor_scalar_add(
                t1[:, :], ne2[0:1, K : 2 * K], es2[0:1, 1:2]
            ).then_inc(dve_sem, 1)                                             # 16
            vector.wait_ge(dve_sem, 16)
            vector.scalar_tensor_tensor(
                pk[:, :], ne2[0:1, 0:K], e0s[0:1, 0:1], t1[:, :],
                op0=mybir.AluOpType.add, op1=mybir.AluOpType.mult,
            ).then_inc(dve_sem, 1)                                             # 17
            vector.wait_ge(dve_sem, 17)
            vector.tensor_reduce(
                mn[:, :], pk[:, :], axis=mybir.AxisListType.X,
                op=mybir.AluOpType.min,
            ).then_inc(dve_sem, 1)                                             # 18
            vector.wait_ge(act_sem, 1)
            vector.tensor_scalar_mul(
                lnee_s[:, :], lgee[:, :], LOG10_SCALE
            ).then_inc(dve_sem, 1)                                             # 19
            vector.wait_ge(dve_sem, 19)
            vector.wait_ge(act_sem, 2)
            vector.scalar_tensor_tensor(
                loss[:, :], lg[:, :], LOG10_SCALE, lnee_s[:, :],
                op0=mybir.AluOpType.mult, op1=mybir.AluOpType.subtract,
            ).then_inc(dve_sem, 1)                                             # 20

        @block.scalar
        def _(scalar):
            # dummy activation on memset scratch preloads the Ln table early
            scalar.wait_ge(dve_sem, 1)
            scalar.activation(dum1[:, :], dum0[:, :], mybir.ActivationFunctionType.Ln)
            scalar.wait_ge(dve_sem, 13)
            scalar.activation(
                lgee[:, :], ee[:, :], mybir.ActivationFunctionType.Ln
            ).then_inc(act_sem, 1)
            scalar.wait_ge(dve_sem, 18)
            scalar.activation(
                lg[:, :], mn[:, :], mybir.ActivationFunctionType.Ln
            ).then_inc(act_sem, 1)

        @block.tensor
        def _(tensor):
            for w in range(N_WAVES):
                b0, b1 = WAVE_EDGES[w], WAVE_EDGES[w + 1]
                tensor.wait_ge(dsem_w[w], 16)
                for blk in range(b0, b1):
                    cols = rint[:, blk * BW : (blk + 1) * BW]
                    tensor.matmul(
                        gp[:, :],
                        cols,
                        cols,
                        start=(blk == 0),
                        stop=(blk == NBLK - 1),
                    ).then_inc(pe_sem, 1)
            tensor.wait_ge(dsem_c, 16)
            tensor.wait_ge(dve_sem, 4)  # pc copied
            for i in range(LBLK):
                tensor.matmul(
                    g10p[:, :],
                    id100[:, i * LBLK : (i + 1) * LBLK],
                    pc[:, i * LBLK : (i + 1) * LBLK],
                    start=(i == 0),
                    stop=(i == LBLK - 1),
                ).then_inc(pe_sem, 1)
            tensor.wait_ge(dve_sem, 5)  # g10b copied (csbb at 3 covered)
            tensor.matmul(
                qte[:, :], g10b[:, :], csbb[0:NSIG, 0 : K + 2]
            ).then_inc(pe_sem, 1)
            tensor.wait_ge(dve_sem, 6)  # g10 f32 copied
            tensor.matmul(rowp[0:1, 0:NSIG], e8col, g10[:, :]).then_inc(pe_sem, 1)
            tensor.matmul(
                rowp[0:1, NSIG : 2 * NSIG], e9col, g10[:, :]
            ).then_inc(pe_sem, 1)
            tensor.matmul(
                rowp[0:1, 2 * NSIG : 3 * NSIG], ones8c, g10[:, :]
            ).then_inc(pe_sem, 1)
            tensor.wait_ge(dve_sem, 9)  # buf8 ready
            tensor.matmul(
                ne2[:, :], csbb[0:M, K + 2 : K + 3], buf8[:, :]
            ).then_inc(pe_sem, 1)

    return nc


_NC_CACHE: bass.Bass | None = None


def _in_maps(est: np.ndarray, mx: np.ndarray) -> list[dict]:
    cst = _const_matrix()
    return [
        {
            "data": _interleave(np.concatenate([est[b], mx[b]], axis=0)),
            "cst": cst,
        }
        for b in range(B)
    ]


def kernel(estimated_sources: np.ndarray, input_mixtures: np.ndarray) -> np.ndarray:
    global _NC_CACHE
    assert estimated_sources.shape == (B, M, T)
    assert input_mixtures.shape == (B, NMIX, T)
    if _NC_CACHE is None:
        _NC_CACHE = _build_kernel()
    nc = _NC_CACHE

    est = np.asarray(estimated_sources, dtype=np.float32)
    mx = np.asarray(input_mixtures, dtype=np.float32)
    res = run_bass_kernel_spmd(nc, _in_maps(est, mx), core_ids=list(range(B)))
    vals = np.array([res.results[b]["loss"][0, 0] for b in range(B)], dtype=np.float32)
    return np.asarray(vals.mean(), dtype=np.float32)
